# revision 11
# baseline (speedup 1.0000x reference)
"""LIF (leaky integrate-and-fire) scan kernel for Trainium2, 8 NeuronCores.

Reference semantics (fp32, T=8 innermost axis):
    mem = 0
    for t in range(T):
        mem = mem * 0.5 + x[..., t]
        s[..., t] = (mem >= 1.0)
        mem = mem * (1.0 - s[..., t])

Strategy (fp32 baseline ~218 us; v3 stt-free ~126 us; v4 CH=4096 ~109 us):
  * int16 fixed-point (scale 2^12): x quantized on the host, membrane M
    int16 on device. Engines compute fp32 internally, so the only error
    sources are x quantization and one int16 rounding of M per step
    (quantum 2^-12): ~1.8e3 spike flips vs the fp32 reference = rel err
    0.0137 < 2e-2 (robust to HW rounding mode; rne/trunc/floor/ceil all
    pass in host simulation).
  * HW-measured op selection (per 2048 elems/partition):
    scalar_tensor_tensor is ALWAYS 1x (2284 ns); tensor_tensor on 16-bit
    is 2x (1225 ns); tensor_scalar 16-bit in/out is 4x (694 ns); the mask
    must be fp16 (int16-out is_lt is pathological, bf16 mixed-TT is 1.3x).
    Per step the DVE runs:
        B: r2_t = ts(M_t is_lt 4096) mult 0.5     fp16 {0,.5}, 4x
        D: M'_t = tt_mult(M_t, r2_t)              int16, 2x = reset AND decay
  * The integrate step (M += y_t) is split between two adders:
      - steps {1,2,4,6}: SWDGE dma_start(accum_op=add) applies the CCE
        inline adder as the strip streams from HBM (verified exact for
        int16; no DVE contention). The RMW costs ~2x SDMA port time, so
        doing ALL steps this way is fabric-bound (~100 us) - and doing
        none is DVE-bound (~95 us). The 4/3 split balances both at ~75.
      - steps {3,5,7}: DVE tensor_tensor add from a plain prefetched
        strip (HWDGE). t=7 is deliberately a TT step so the kernel tail
        has no DMA round-trip in it.
  * All per-engine instruction streams are emitted T-STEP-MAJOR across
    the 4 chunks: the SWDGE FIFO is strictly in-order, so chunk-major
    emission blocks the queue head on each step's TT dependency and
    serializes everything (measured 263 us).
  * Packed u8 output (32x less output traffic): PE accumulates 8 matmuls
    with diagonal fp16 weights -2^(t+1) over the r2 planes into PSUM:
    psum = -sum_t 2^t r_t = packed_byte - 255; ACT adds 255 during the
    PSUM -> SBUF u8 copy. PSUM only holds 2 chunks' banks, so chunks 0/1
    pack inline and chunks 2/3's matmuls are emitted after chunk 0/1's
    copies (their fp16 masks park in SBUF; inline emission would deadlock
    the in-order PE queue on PSUM reuse).
  * Per-core HBM traffic: 16 MiB in + 1 MiB out (~50 us at 358 GB/s).
"""

import numpy as np

import concourse.bass as bass
import concourse.tile as tile
from concourse import bacc, mybir
from concourse.bass_utils import run_bass_kernel_spmd

P = 128          # SBUF partitions
T = 8            # timesteps (innermost axis of the original input)
NPB = 8192       # neurons per partition per core: 8*128*32*32 / 128
CH = 2048        # neurons per chunk (per partition)
NCH = NPB // CH
PSB = 512        # psum bank free size (fp32)
NB = CH // PSB   # psum banks per chunk
NPACK = 2        # chunks whose pack-matmuls run inline (PSUM = NPACK*NB banks)

ACCUM_STEPS = {1, 2, 4, 6}   # integrate via DMA-CCE; the rest via DVE TT

SCALE = 4096.0   # fixed-point scale 2^12
THR = 4096.0     # threshold 1.0 in scaled units
N_CORES = 8

F32 = mybir.dt.float32
I16 = mybir.dt.int16
U8 = mybir.dt.uint8
F16 = mybir.dt.float16

Alu = mybir.AluOpType
Act = mybir.ActivationFunctionType


def _build() -> bass.Bass:
    nc = bacc.Bacc("TRN2", target_bir_lowering=False, debug=False)
    x = nc.dram_tensor("x", [P, T, NPB], I16, kind="ExternalInput").ap()
    w = nc.dram_tensor("w", [P, T * P], F16, kind="ExternalInput").ap()
    y = nc.dram_tensor("y", [P, NPB], U8, kind="ExternalOutput").ap()

    with tile.TileContext(nc) as tc:
        with (
            tc.tile_pool(name="mem", bufs=14) as mem,
            tc.tile_pool(name="msk", bufs=20) as msk,
            tc.tile_pool(name="xin", bufs=8) as xin,
            tc.tile_pool(name="acc", bufs=2) as accp,
            tc.tile_pool(name="wts", bufs=1) as wts,
            tc.tile_pool(name="ps", bufs=1, space="PSUM") as psp,
        ):
            wt = wts.tile([P, T * P], F16, tag="w", name="wt")
            nc.sync.dma_start(wt[:], w[:, :])
            bias_p = wts.tile([P, 1], F32, tag="bp", name="bias_p")
            nc.vector.memset(bias_p[:], 255.0)

            ps = {}      # (c, b) -> psum tile
            r2s = {}     # (c, t) -> mask tile
            strips = {}  # (c, t) -> prefetched input strip (TT steps)
            cur = [None] * NCH

            def pack_mms(c, t):
                for b in range(NB):
                    nc.tensor.matmul(
                        ps[(c, b)][:],
                        wt[:, t * P : (t + 1) * P],
                        r2s[(c, t)][:, b * PSB : (b + 1) * PSB],
                        start=(t == 0),
                        stop=(t == T - 1),
                    )

            def pack_copy(c):
                acc = accp.tile([P, CH], U8, tag="a", name=f"a{c}")
                for b in range(NB):
                    nc.scalar.activation(
                        acc[:, b * PSB : (b + 1) * PSB],
                        ps[(c, b)][:],
                        Act.Identity,
                        bias=bias_p[:],
                        scale=1.0,
                    )
                nc.sync.dma_start(y[:, c * CH : (c + 1) * CH], acc[:])

            def prefetch(c, t):
                st = xin.tile([P, CH], I16, tag="x", name=f"x{c}_{t}")
                nc.sync.dma_start(st[:], x[:, t, c * CH : (c + 1) * CH])
                strips[(c, t)] = st

            for c in range(NCH):
                for b in range(NB):
                    ps[(c, b)] = psp.tile(
                        [P, PSB], F32, tag=f"ps{b + (c % NPACK) * NB}",
                        name=f"ps{c}_{b}",
                    )

            # t-step-major main loop across all chunks
            for t in range(T):
                for c in range(NCH):
                    lo = c * CH
                    if t == 0:
                        cur[c] = mem.tile([P, CH], I16, tag="m", name=f"m{c}_0")
                        nc.sync.dma_start(cur[c][:], x[:, 0, lo : lo + CH])
                    elif t in ACCUM_STEPS:
                        # M_t = M'_{t-1} + y_t via the DMA CCE inline adder
                        nc.gpsimd.dma_start(
                            cur[c][:], x[:, t, lo : lo + CH], accum_op=Alu.add
                        )
                    else:
                        nxt = mem.tile([P, CH], I16, tag="m", name=f"a{c}_{t}")
                        nc.vector.tensor_tensor(
                            nxt[:], cur[c][:], strips.pop((c, t))[:], Alu.add
                        )
                        cur[c] = nxt
                    # prefetch the strip for the next TT-add step
                    for tf in (t + 1, t + 2):
                        if tf < T and tf not in ACCUM_STEPS and (c, tf) not in strips:
                            prefetch(c, tf)
                            break
                    r2 = msk.tile([P, CH], F16, tag="r", name=f"r{c}_{t}")
                    nc.vector.tensor_scalar(
                        r2[:], cur[c][:], THR, 0.5, Alu.is_lt, Alu.mult
                    )
                    r2s[(c, t)] = r2
                    if c < NPACK:
                        pack_mms(c, t)
                    if t < T - 1:
                        nxt = mem.tile([P, CH], I16, tag="m", name=f"d{c}_{t}")
                        nc.vector.tensor_tensor(
                            nxt[:], cur[c][:], r2[:], Alu.mult
                        )
                        cur[c] = nxt

            for c in range(NPACK):
                pack_copy(c)
            for c in range(NPACK, NCH):
                for t in range(T):
                    pack_mms(c, t)
            for c in range(NPACK, NCH):
                pack_copy(c)
    nc.compile()
    return nc


_NC_CACHE: bass.Bass | None = None


def _get_nc() -> bass.Bass:
    global _NC_CACHE
    if _NC_CACHE is None:
        _NC_CACHE = _build()
    return _NC_CACHE


def _weights() -> np.ndarray:
    # W_t = -2^(t+1) * I, laid out as [P, T*P] (lhsT slices [128, 128] per t).
    # psum = sum_t W_t^T r2_t = -sum_t 2^t r_t = packed_byte - 255.
    wf = np.zeros((P, T * P), dtype=np.float32)
    for t in range(T):
        wf[:, t * P : (t + 1) * P][np.arange(P), np.arange(P)] = -(2.0 ** (t + 1))
    return wf.astype(np.float16)


def _run(X: np.ndarray, **spmd_kwargs):
    assert X.shape == (64, 128, 32, 32, 8), X.shape
    X = np.asarray(X, dtype=np.float32)
    per_core = 64 // N_CORES
    q = np.clip(np.rint(X * SCALE), -32768.0, 32767.0).astype(np.int16)
    # [core, p, n, t] -> t-major [core, p, t, n], contiguous per core
    qt = np.ascontiguousarray(
        q.reshape(N_CORES, P, NPB, T).transpose(0, 1, 3, 2)
    )
    wnp = _weights()
    in_maps = [{"x": qt[i], "w": wnp} for i in range(N_CORES)]
    res = run_bass_kernel_spmd(
        _get_nc(), in_maps, core_ids=list(range(N_CORES)), **spmd_kwargs
    )
    out = np.empty_like(X)
    for i, r in enumerate(res.results):
        packed = r["y"].reshape(P, NPB, 1).astype(np.uint8)
        bits = np.unpackbits(packed, axis=2, bitorder="little")  # [P, NPB, 8]
        out[i * per_core : (i + 1) * per_core] = bits.astype(np.float32).reshape(
            per_core, 128, 32, 32, 8
        )
    return out, res


def kernel(X: np.ndarray) -> np.ndarray:
    out, _ = _run(X)
    return out


# revision 16
# speedup vs baseline: 1.1343x; 1.1343x over previous
"""LIF (leaky integrate-and-fire) scan kernel for Trainium2, 8 NeuronCores.

Reference semantics (fp32, T=8 innermost axis):
    mem = 0
    for t in range(T):
        mem = mem * 0.5 + x[..., t]
        s[..., t] = (mem >= 1.0)
        mem = mem * (1.0 - s[..., t])

Strategy (fp32 baseline ~218 us; v3 ~126 us; v4 ~109 us):
  * int16 fixed-point (scale 2^12): x quantized on the host, membrane M
    int16 on device. Engines compute fp32 internally, so the only error
    sources are x quantization and one int16 rounding of M per step
    (quantum 2^-12): 1841 spike flips vs the fp32 reference = rel err
    0.0137 < 2e-2 (robust to HW rounding mode; rne/trunc/floor/ceil all
    pass in host simulation).
  * HW-measured op selection (per 2048 elems/partition):
    scalar_tensor_tensor is ALWAYS 1x (2284 ns) regardless of dtype;
    tensor_tensor on 16-bit is 2x (1225 ns); tensor_scalar 16-bit in/out
    is 4x (694 ns); the mask must be fp16 (int16-out is_lt is
    pathological at 3529 ns, bf16 mixed-TT drops to 1628 ns). The update
    avoids stt entirely and fuses decay+reset into one tensor_tensor via
    a {0, 0.5}-valued mask:
        A: M_t  = tt_add(M'_{t-1}, y_t)            int16, 2x   (y = x*4096)
        B: r2_t = ts(M_t is_lt 4096) mult 0.5      fp16 {0,.5}, 4x
        D: M'_t = tt_mult(M_t, r2_t)               int16, 2x = reset AND decay
    (DMA-CCE accumulate was tried for A: exact for int16, but its RMW
    doubles SDMA port traffic and its ~5.5 us round-trip needs >=4
    in-flight chunk chains to hide, which PSUM (8 banks = 2 chunks of
    packing state) cannot support — measured 127-263 us. Pure-DVE wins.)
  * Packed u8 output (32x less output traffic): PE accumulates 8 matmuls
    with diagonal fp16 weights -2^(t+1) over the r2 planes into PSUM:
    psum = -sum_t 2^t r_t = packed_byte - 255; ACT adds 255 during the
    PSUM -> SBUF u8 copy. byte bit t = spike at step t.
  * Per-core HBM traffic: 16 MiB in + 1 MiB out (~50 us at 358 GB/s).
  * Uneven chunks [4096, 2048, 2048]: the big chunks amortize DVE
    per-instruction overhead; the final 2048 chunk halves the kernel tail
    (last mask -> 4 matmuls -> copies -> out-DMA). Input arrives as
    per-timestep 0.5-1 MiB DMAs, and the t=0/t=1 strips of chunk 0 are
    issued first so compute starts ~3 us in.

Per-core layout: data-parallel over the leading dim (64 -> 8 per core),
t-major strips [128 partitions, T=8, 8192 neurons]; all compute touches
contiguous strips (keeps DVE 2x/4x modes + dense DMA descriptors).
"""

import numpy as np

import concourse.bass as bass
import concourse.tile as tile
from concourse import bacc, mybir
from concourse.bass_utils import run_bass_kernel_spmd

P = 128          # SBUF partitions
T = 8            # timesteps (innermost axis of the original input)
NPB = 8192       # neurons per partition per core: 8*128*32*32 / 128
CHUNKS = (4096, 2048, 2048)
PSB = 512        # psum bank free size (fp32)

SCALE = 4096.0   # fixed-point scale 2^12
THR = 4096.0     # threshold 1.0 in scaled units
N_CORES = 8

F32 = mybir.dt.float32
I16 = mybir.dt.int16
U8 = mybir.dt.uint8
F16 = mybir.dt.float16

Alu = mybir.AluOpType
Act = mybir.ActivationFunctionType


def _build() -> bass.Bass:
    nc = bacc.Bacc("TRN2", target_bir_lowering=False, debug=False)
    x = nc.dram_tensor("x", [P, T, NPB], I16, kind="ExternalInput").ap()
    w = nc.dram_tensor("w", [P, T * P], F16, kind="ExternalInput").ap()
    y = nc.dram_tensor("y", [P, NPB], U8, kind="ExternalOutput").ap()

    with tile.TileContext(nc) as tc:
        with (
            tc.tile_pool(name="xin", bufs=10) as xin,
            tc.tile_pool(name="mem", bufs=6) as mem,
            tc.tile_pool(name="msk", bufs=6) as msk,
            tc.tile_pool(name="acc", bufs=2) as accp,
            tc.tile_pool(name="wts", bufs=1) as wts,
            tc.tile_pool(name="ps", bufs=1, space="PSUM") as psp,
        ):
            # chunk 0's first strips lead the DMA queue so compute starts asap
            first = xin.tile([P, 2, 4096], I16, tag="x0", name="first", bufs=1)
            nc.sync.dma_start(first[:], x[:, 0:2, 0:4096])
            wt = wts.tile([P, T * P], F16, tag="w", name="wt")
            nc.sync.dma_start(wt[:], w[:, :])
            bias_p = wts.tile([P, 1], F32, tag="bp", name="bias_p")
            nc.vector.memset(bias_p[:], 255.0)

            lo = 0
            bank0 = 0
            for c, CH in enumerate(CHUNKS):
                NB = CH // PSB
                xs = []
                for t in range(T):
                    if c == 0 and t < 2:
                        xs.append(first[:, t, :])
                        continue
                    st = xin.tile([P, CH], I16, tag="x", name=f"x{c}_{t}")
                    nc.sync.dma_start(st[:], x[:, t, lo : lo + CH])
                    xs.append(st[:])
                ps = [
                    psp.tile(
                        [P, PSB], F32, tag=f"ps{(bank0 + b) % 8}",
                        name=f"ps{c}_{b}",
                    )
                    for b in range(NB)
                ]
                acc = accp.tile([P, CH], U8, tag="a", name=f"a{c}")
                cur = xs[0]  # M_0 = y_0 (mem starts at 0): alias, no copy
                for t in range(T):
                    if t > 0:
                        nxt = mem.tile([P, CH], I16, tag="m", name=f"m{c}_{t}")
                        nc.vector.tensor_tensor(nxt[:], prev[:], xs[t], Alu.add)
                        cur = nxt[:]
                    r2 = msk.tile([P, CH], F16, tag="r", name=f"r{c}_{t}")
                    nc.vector.tensor_scalar(
                        r2[:], cur, THR, 0.5, Alu.is_lt, Alu.mult
                    )
                    for b in range(NB):
                        nc.tensor.matmul(
                            ps[b][:],
                            wt[:, t * P : (t + 1) * P],
                            r2[:, b * PSB : (b + 1) * PSB],
                            start=(t == 0),
                            stop=(t == T - 1),
                        )
                    if t < T - 1:
                        rst = mem.tile([P, CH], I16, tag="m", name=f"d{c}_{t}")
                        nc.vector.tensor_tensor(rst[:], cur, r2[:], Alu.mult)
                        prev = rst
                for b in range(NB):
                    nc.scalar.activation(
                        acc[:, b * PSB : (b + 1) * PSB],
                        ps[b][:],
                        Act.Identity,
                        bias=bias_p[:],
                        scale=1.0,
                    )
                    if b % 4 == 3:
                        nc.sync.dma_start(
                            y[:, lo + (b - 3) * PSB : lo + (b + 1) * PSB],
                            acc[:, (b - 3) * PSB : (b + 1) * PSB],
                        )
                lo += CH
                bank0 += NB
    nc.compile()
    return nc


_NC_CACHE: bass.Bass | None = None


def _get_nc() -> bass.Bass:
    global _NC_CACHE
    if _NC_CACHE is None:
        _NC_CACHE = _build()
    return _NC_CACHE


def _weights() -> np.ndarray:
    # W_t = -2^(t+1) * I, laid out as [P, T*P] (lhsT slices [128, 128] per t).
    # psum = sum_t W_t^T r2_t = -sum_t 2^t r_t = packed_byte - 255.
    wf = np.zeros((P, T * P), dtype=np.float32)
    for t in range(T):
        wf[:, t * P : (t + 1) * P][np.arange(P), np.arange(P)] = -(2.0 ** (t + 1))
    return wf.astype(np.float16)


def _run(X: np.ndarray, **spmd_kwargs):
    assert X.shape == (64, 128, 32, 32, 8), X.shape
    X = np.asarray(X, dtype=np.float32)
    per_core = 64 // N_CORES
    q = np.clip(np.rint(X * SCALE), -32768.0, 32767.0).astype(np.int16)
    # [core, p, n, t] -> t-major [core, p, t, n], contiguous per core
    qt = np.ascontiguousarray(
        q.reshape(N_CORES, P, NPB, T).transpose(0, 1, 3, 2)
    )
    wnp = _weights()
    in_maps = [{"x": qt[i], "w": wnp} for i in range(N_CORES)]
    res = run_bass_kernel_spmd(
        _get_nc(), in_maps, core_ids=list(range(N_CORES)), **spmd_kwargs
    )
    out = np.empty_like(X)
    for i, r in enumerate(res.results):
        packed = r["y"].reshape(P, NPB, 1).astype(np.uint8)
        bits = np.unpackbits(packed, axis=2, bitorder="little")  # [P, NPB, 8]
        out[i * per_core : (i + 1) * per_core] = bits.astype(np.float32).reshape(
            per_core, 128, 32, 32, 8
        )
    return out, res


def kernel(X: np.ndarray) -> np.ndarray:
    out, _ = _run(X)
    return out


# revision 18
# speedup vs baseline: 1.1818x; 1.0419x over previous
"""LIF (leaky integrate-and-fire) scan kernel for Trainium2, 8 NeuronCores.

Reference semantics (fp32, T=8 innermost axis):
    mem = 0
    for t in range(T):
        mem = mem * 0.5 + x[..., t]
        s[..., t] = (mem >= 1.0)
        mem = mem * (1.0 - s[..., t])

Strategy (fp32 baseline ~218 us; v3 ~126 us; v4 ~109 us):
  * int16 fixed-point (scale 2^12): x quantized on the host, membrane M
    int16 on device. Engines compute fp32 internally, so the only error
    sources are x quantization and one int16 rounding of M per step
    (quantum 2^-12): 1841 spike flips vs the fp32 reference = rel err
    0.0137 < 2e-2 (robust to HW rounding mode; rne/trunc/floor/ceil all
    pass in host simulation).
  * HW-measured op selection (per 2048 elems/partition):
    scalar_tensor_tensor is ALWAYS 1x (2284 ns) regardless of dtype;
    tensor_tensor on 16-bit is 2x (1225 ns); tensor_scalar 16-bit in/out
    is 4x (694 ns); the mask must be fp16 (int16-out is_lt is
    pathological at 3529 ns, bf16 mixed-TT drops to 1628 ns). The update
    avoids stt entirely and fuses decay+reset into one tensor_tensor via
    a {0, 0.5}-valued mask:
        A: M_t  = tt_add(M'_{t-1}, y_t)            int16, 2x   (y = x*4096)
        B: r2_t = ts(M_t is_lt 4096) mult 0.5      fp16 {0,.5}, 4x
        D: M'_t = tt_mult(M_t, r2_t)               int16, 2x = reset AND decay
    (DMA-CCE accumulate was tried for A: exact for int16, but its RMW
    doubles SDMA port traffic and its ~5.5 us round-trip needs >=4
    in-flight chunk chains to hide, which PSUM (8 banks = 2 chunks of
    packing state) cannot support — measured 127-263 us. Pure-DVE wins.)
  * Packed u8 output (32x less output traffic): PE accumulates 8 matmuls
    with diagonal fp16 weights -2^(t+1) over the r2 planes into PSUM:
    psum = -sum_t 2^t r_t = packed_byte - 255; ACT adds 255 during the
    PSUM -> SBUF u8 copy. byte bit t = spike at step t.
  * Per-core HBM traffic: 16 MiB in + 1 MiB out (~50 us at 358 GB/s).
  * Uneven chunks [4096, 2048, 2048]: the big chunks amortize DVE
    per-instruction overhead; the final 2048 chunk halves the kernel tail
    (last mask -> 4 matmuls -> copies -> out-DMA). Input arrives as
    per-timestep 0.5-1 MiB DMAs, and the t=0/t=1 strips of chunk 0 are
    issued first so compute starts ~3 us in.

Per-core layout: data-parallel over the leading dim (64 -> 8 per core),
t-major strips [128 partitions, T=8, 8192 neurons]; all compute touches
contiguous strips (keeps DVE 2x/4x modes + dense DMA descriptors).
"""

import numpy as np

import concourse.bass as bass
import concourse.tile as tile
from concourse import bacc, mybir
from concourse.bass_utils import run_bass_kernel_spmd

P = 128          # SBUF partitions
T = 8            # timesteps (innermost axis of the original input)
NPB = 8192       # neurons per partition per core: 8*128*32*32 / 128
CHUNKS = (4096, 4096)
PSB = 512        # psum bank free size (fp32)

SCALE = 4096.0   # fixed-point scale 2^12
THR = 4096.0     # threshold 1.0 in scaled units
N_CORES = 8

F32 = mybir.dt.float32
I16 = mybir.dt.int16
U8 = mybir.dt.uint8
F16 = mybir.dt.float16

Alu = mybir.AluOpType
Act = mybir.ActivationFunctionType


def _build() -> bass.Bass:
    nc = bacc.Bacc("TRN2", target_bir_lowering=False, debug=False)
    x = nc.dram_tensor("x", [P, T, NPB], I16, kind="ExternalInput").ap()
    w = nc.dram_tensor("w", [P, T * P], F16, kind="ExternalInput").ap()
    y = nc.dram_tensor("y", [P, NPB], U8, kind="ExternalOutput").ap()

    with tile.TileContext(nc) as tc:
        with (
            tc.tile_pool(name="xin", bufs=10) as xin,
            tc.tile_pool(name="mem", bufs=6) as mem,
            tc.tile_pool(name="msk", bufs=6) as msk,
            tc.tile_pool(name="acc", bufs=2) as accp,
            tc.tile_pool(name="wts", bufs=1) as wts,
            tc.tile_pool(name="ps", bufs=1, space="PSUM") as psp,
        ):
            wt = wts.tile([P, T * P], F16, tag="w", name="wt")
            nc.sync.dma_start(wt[:], w[:, :])
            bias_p = wts.tile([P, 1], F32, tag="bp", name="bias_p")
            nc.vector.memset(bias_p[:], 255.0)

            lo = 0
            bank0 = 0
            for c, CH in enumerate(CHUNKS):
                NB = CH // PSB
                xs = []
                for t in range(T):
                    st = xin.tile([P, CH], I16, tag="x", name=f"x{c}_{t}")
                    nc.sync.dma_start(st[:], x[:, t, lo : lo + CH])
                    xs.append(st[:])
                ps = [
                    psp.tile(
                        [P, PSB], F32, tag=f"ps{(bank0 + b) % 8}",
                        name=f"ps{c}_{b}",
                    )
                    for b in range(NB)
                ]
                acc = accp.tile([P, CH], U8, tag="a", name=f"a{c}")
                cur = xs[0]  # M_0 = y_0 (mem starts at 0): alias, no copy
                for t in range(T):
                    if t > 0:
                        nxt = mem.tile([P, CH], I16, tag="m", name=f"m{c}_{t}")
                        nc.vector.tensor_tensor(nxt[:], prev[:], xs[t], Alu.add)
                        cur = nxt[:]
                    r2 = msk.tile([P, CH], F16, tag="r", name=f"r{c}_{t}")
                    nc.vector.tensor_scalar(
                        r2[:], cur, THR, 0.5, Alu.is_lt, Alu.mult
                    )
                    for b in range(NB):
                        nc.tensor.matmul(
                            ps[b][:],
                            wt[:, t * P : (t + 1) * P],
                            r2[:, b * PSB : (b + 1) * PSB],
                            start=(t == 0),
                            stop=(t == T - 1),
                        )
                    if t < T - 1:
                        rst = mem.tile([P, CH], I16, tag="m", name=f"d{c}_{t}")
                        nc.vector.tensor_tensor(rst[:], cur, r2[:], Alu.mult)
                        prev = rst
                for b in range(NB):
                    nc.scalar.activation(
                        acc[:, b * PSB : (b + 1) * PSB],
                        ps[b][:],
                        Act.Identity,
                        bias=bias_p[:],
                        scale=1.0,
                    )
                    if b % 4 == 3:
                        nc.sync.dma_start(
                            y[:, lo + (b - 3) * PSB : lo + (b + 1) * PSB],
                            acc[:, (b - 3) * PSB : (b + 1) * PSB],
                        )
                lo += CH
                bank0 += NB
    nc.compile()
    return nc


_NC_CACHE: bass.Bass | None = None


def _get_nc() -> bass.Bass:
    global _NC_CACHE
    if _NC_CACHE is None:
        _NC_CACHE = _build()
    return _NC_CACHE


def _weights() -> np.ndarray:
    # W_t = -2^(t+1) * I, laid out as [P, T*P] (lhsT slices [128, 128] per t).
    # psum = sum_t W_t^T r2_t = -sum_t 2^t r_t = packed_byte - 255.
    wf = np.zeros((P, T * P), dtype=np.float32)
    for t in range(T):
        wf[:, t * P : (t + 1) * P][np.arange(P), np.arange(P)] = -(2.0 ** (t + 1))
    return wf.astype(np.float16)


def _run(X: np.ndarray, **spmd_kwargs):
    assert X.shape == (64, 128, 32, 32, 8), X.shape
    X = np.asarray(X, dtype=np.float32)
    per_core = 64 // N_CORES
    q = np.clip(np.rint(X * SCALE), -32768.0, 32767.0).astype(np.int16)
    # [core, p, n, t] -> t-major [core, p, t, n], contiguous per core
    qt = np.ascontiguousarray(
        q.reshape(N_CORES, P, NPB, T).transpose(0, 1, 3, 2)
    )
    wnp = _weights()
    in_maps = [{"x": qt[i], "w": wnp} for i in range(N_CORES)]
    res = run_bass_kernel_spmd(
        _get_nc(), in_maps, core_ids=list(range(N_CORES)), **spmd_kwargs
    )
    out = np.empty_like(X)
    for i, r in enumerate(res.results):
        packed = r["y"].reshape(P, NPB, 1).astype(np.uint8)
        bits = np.unpackbits(packed, axis=2, bitorder="little")  # [P, NPB, 8]
        out[i * per_core : (i + 1) * per_core] = bits.astype(np.float32).reshape(
            per_core, 128, 32, 32, 8
        )
    return out, res


def kernel(X: np.ndarray) -> np.ndarray:
    out, _ = _run(X)
    return out
